# revision 42
# baseline (speedup 1.0000x reference)
"""Trainium2 Bass kernel for nn_MultiHeadSelfAttention_22668837388979.

Sharding: 8 cores = batch(2) x head-groups(4).  Each core handles one batch
element and 4 of the 16 heads:
  - QKV projection (bf16 matmuls, f32 accum) for its heads
  - causal ghost-softmax attention
  - row-parallel output projection partials (one per head pair)
Host sums the per-core, per-pair partials and adds Wo_b.

Ghost softmax identity used on device (no max-subtraction pass needed):
  S = exp(s - m) / (sum exp(s - m) + g)  ==  z / (sum z + g * e^m),  z = exp(s)
e^m is recovered from the z tiles themselves: e^m = max_k z.  Per-head the
max over k is a cheap chain of bf16 DVE max-folds across the k-tile slices
(free axis) followed by one GpSimd partition_all_reduce (partition axis),
and 16 tiny PE transposes bring the row into [128,16] column layout for the
per-row denominator math.  sum z comes for free from a ones-column appended
to V in the S@V matmul.

Scheduling (from HAM throttle analysis): the PE clamps to 50% util whenever
the recent *instruction* mix is K=64-heavy (row-group pairing does not help
the HAM state, only throughput within it), and re-ramping costs ~20us.  So
every K=64 score matmul is issued interleaved with K=128 work (V proj, the
second half of QKV, S@V, output projection), the PE is pre-heated during
the initial input DMA, and DMAs are batched into few descriptors
(sync-engine issue costs ~0.65us each).  The z pipeline is gated by the
scalar engine's exp throughput, so the interleave ratio keeps the PE fed
with dense work while exps drain.
"""

import math
from collections import deque

import numpy as np
import ml_dtypes

EMBED = 1024
NHEAD = 16
D = 64
B = 2
S = 2048
HPC = 4          # heads per core
NCORES = 8
P = 128
NEG = -1.0e9
N_PRE = 60       # pre-heat matmuls at kernel start (PE idles during input DMA)

_prog_cache = {}


def _build_program():
    import concourse.bass as bass
    import concourse.tile as tile
    from concourse import bacc, mybir, bass_isa

    f32 = mybir.dt.float32
    bf16 = mybir.dt.bfloat16
    AF = mybir.ActivationFunctionType
    ALU = mybir.AluOpType
    AX = mybir.AxisListType

    nc = bacc.Bacc("TRN2", target_bir_lowering=False, debug=False)

    xT = nc.dram_tensor("xT", [EMBED, S], bf16, kind="ExternalInput").ap()
    wqkT = nc.dram_tensor("wqkT", [EMBED, 2 * HPC * D], bf16, kind="ExternalInput").ap()
    wvT = nc.dram_tensor("wvT", [EMBED, HPC * D], bf16, kind="ExternalInput").ap()
    woT = nc.dram_tensor("woT", [HPC * D, EMBED], bf16, kind="ExternalInput").ap()
    qkb = nc.dram_tensor("qkb", [P, 4], f32, kind="ExternalInput").ap()
    vb = nc.dram_tensor("vb", [1, HPC * D], bf16, kind="ExternalInput").ap()
    gco = nc.dram_tensor("gco", [P, HPC], f32, kind="ExternalInput").ap()
    idbf = nc.dram_tensor("idbf", [P, P], bf16, kind="ExternalInput").ap()
    idf = nc.dram_tensor("idf", [P, P], f32, kind="ExternalInput").ap()
    trLf = nc.dram_tensor("trLf", [P, P], f32, kind="ExternalInput").ap()
    out0 = nc.dram_tensor("out0", [S // P // 2, 2, P, 2, 512], bf16,
                          kind="ExternalOutput").ap()
    out1 = nc.dram_tensor("out1", [S // P // 2, 2, P, 2, 512], bf16,
                          kind="ExternalOutput").ap()

    with tile.TileContext(nc) as tc:
        _body(tc, bass, mybir, bass_isa, f32, bf16, AF, ALU, AX,
              xT, wqkT, wvT, woT, qkb, vb, gco, idbf, idf, trLf,
              out0, out1)

    nc.compile()
    return nc


def _body(tc, bass, mybir, bass_isa, f32, bf16, AF, ALU, AX,
          xT, wqkT, wvT, woT, qkb, vb, gco, idbf, idf, trLf,
          out0, out1):
    from contextlib import ExitStack
    nc = tc.nc
    NQT = S // P                 # 16 query tiles per head
    ctx = ExitStack()

    const = ctx.enter_context(tc.tile_pool(name="const", bufs=1))
    xpool = ctx.enter_context(tc.tile_pool(name="xpool", bufs=1))
    wpool = ctx.enter_context(tc.tile_pool(name="wpool", bufs=1))
    qkt = ctx.enter_context(tc.tile_pool(name="qkt", bufs=1))
    vsb = ctx.enter_context(tc.tile_pool(name="vsb", bufs=1))
    # z tiles: one big buffer per (pair, H-half); slices addressed by offset
    W_H0 = sum(1024 - 128 * kj for kj in range(8))              # 4608
    W_H1 = sum(min(1024, 2048 - 128 * kj) for kj in range(16))  # 12800
    zp0 = ctx.enter_context(tc.tile_pool(name="zp0", bufs=1))
    zp1 = ctx.enter_context(tc.tile_pool(name="zp1", bufs=1))
    accp = ctx.enter_context(tc.tile_pool(name="accp", bufs=1))
    parp = ctx.enter_context(tc.tile_pool(name="parp", bufs=1))
    spool = ctx.enter_context(tc.tile_pool(name="spool", bufs=2))
    at = ctx.enter_context(tc.tile_pool(name="at", bufs=1))
    osb = ctx.enter_context(tc.tile_pool(name="osb", bufs=2))
    dscr = ctx.enter_context(tc.tile_pool(name="dscr", bufs=2, space="DRAM"))

    uep = ctx.enter_context(tc.tile_pool(name="uep", bufs=1))
    drp = ctx.enter_context(tc.tile_pool(name="drp", bufs=1))

    ps_s = ctx.enter_context(tc.tile_pool(name="ps_s", bufs=2, space="PSUM"))
    ps_d = ctx.enter_context(tc.tile_pool(name="ps_d", bufs=2, space="PSUM"))
    ps_u = ctx.enter_context(tc.tile_pool(name="ps_u", bufs=2, space="PSUM"))

    def heater(n=16):
        """Dense K=128 matmul burst to keep/restore the PE HAM clock."""
        hp = ps_d.tile([P, 512], f32, tag="d", name="hp")
        for i in range(n):
            nc.tensor.matmul(hp, id_bf, wq_heat, start=True, stop=True)
        nc.vector.tensor_copy(heat_sink, hp[:, 0:1])

    # ---- constants built on-device (no DMA: pre-heat starts immediately) -
    tmp1b = const.tile([P, P], bf16)
    nc.vector.memset(tmp1b, 1.0)
    id_bf = const.tile([P, P], bf16)
    nc.gpsimd.affine_select(id_bf, tmp1b, [[1, P]], ALU.is_equal, 0.0,
                            base=0, channel_multiplier=-1)
    tmp1f = const.tile([P, P], f32)
    nc.vector.memset(tmp1f, 1.0)
    id_f = const.tile([P, P], f32)
    nc.gpsimd.affine_select(id_f, tmp1f, [[1, P]], ALU.is_equal, 0.0,
                            base=0, channel_multiplier=-1)
    tmp0f = const.tile([P, P], f32)
    nc.vector.memset(tmp0f, 0.0)
    trlf_sb = const.tile([P, P], f32)
    nc.gpsimd.affine_select(trlf_sb, tmp0f, [[1, P]], ALU.is_ge, NEG,
                            base=0, channel_multiplier=-1)
    ones1 = const.tile([1, P], bf16)
    nc.vector.memset(ones1, 1.0)
    heat_sink = const.tile([P, 1], f32)
    wq_heat = const.tile([P, 512], bf16)
    nc.vector.memset(wq_heat, 0.0)

    # pre-heat the PE while the big input DMAs stream in
    heater(N_PRE)

    qkb_sb = const.tile([P, 4], f32)
    nc.sync.dma_start(qkb_sb, qkb)
    vb_sb = const.tile([1, HPC * D], bf16)
    nc.sync.dma_start(vb_sb, vb)
    gco_sb = const.tile([P, HPC], f32)
    nc.sync.dma_start(gco_sb, gco)

    # ---- bulk inputs (single descriptor each; sync-engine issue is dear) -
    wqkT_sb = wpool.tile([P, 8, 2 * HPC * D], bf16)
    nc.sync.dma_start(wqkT_sb, wqkT.rearrange("(e p) c -> p e c", p=P))
    xT_a = xpool.tile([P, 8, S // 2], bf16, tag="xa", name="xT_a")
    nc.sync.dma_start(xT_a[:, 0:4, :],
                      xT[0:4 * P, 0:S // 2].rearrange("(e p) s -> p e s", p=P))
    nc.sync.dma_start(xT_a[:, 4:8, :],
                      xT[4 * P:, 0:S // 2].rearrange("(e p) s -> p e s", p=P))
    xT_b = xpool.tile([P, 8, S // 2], bf16, tag="xb", name="xT_b")
    nc.sync.dma_start(xT_b, xT[:, S // 2:].rearrange("(e p) s -> p e s", p=P))

    def xT_at(qc):
        return (xT_a if qc < 2 else xT_b), (qc if qc < 2 else qc - 2)
    wvT_sb = wpool.tile([P, 8, HPC * D], bf16)
    nc.sync.dma_start(wvT_sb, wvT.rearrange("(e p) c -> p e c", p=P))
    woT_sb = wpool.tile([P, 2, EMBED], bf16)
    nc.sync.dma_start(woT_sb, woT.rearrange("(b p) e -> p b e", p=P))

    QKT_sb = qkt.tile([P, 4, S], bf16)
    V_sb = vsb.tile([P, NQT, HPC, D + 1], bf16)
    nc.vector.memset(V_sb[:, :, :, D:D + 1], 1.0)
    AT_sb = at.tile([P, 2, S], bf16)   # A^T stacked: partitions = head%2*64+d
    zbig = {0: zp0.tile([P, 2, W_H0], bf16, name="zbig0"),
            1: zp1.tile([P, 2, W_H1], bf16, name="zbig1")}
    zoff = {}                     # (H, kj) -> (offset, rs, rw)
    for H in range(2):
        off = 0
        for kj in range(16 if H else 8):
            rs = max(kj * P, H * 1024)
            rw = H * 1024 + 1024 - rs
            zoff[(H, kj)] = (off, rs, rw)
            off += rw

    # ---- dense (K=128) unit machinery -----------------------------------
    dense_q = deque()

    def drain(cols):
        # Issue dense cover; if none is queued, burn a short K=128 heater
        # burst instead so the HAM never sees a K=64-only window.
        while cols > 0:
            if dense_q:
                cols -= dense_q.popleft()()
            else:
                hp = ps_d.tile([P, 512], f32, tag="d", name="hf")
                for _ in range(4):
                    nc.tensor.matmul(hp, id_bf, wq_heat, start=True, stop=True)
                nc.vector.tensor_copy(heat_sink, hp[:, 0:1])
                cols -= 2048

    def drain_all():
        while dense_q:
            dense_q.popleft()()

    def qkv_unit(fb, qc):
        def u():
            ps = ps_d.tile([P, 512], f32, tag="d", name="qkvps")
            xt, qc2 = xT_at(qc)
            for e in range(8):
                nc.tensor.matmul(
                    ps,
                    wqkT_sb[:, e, fb * P:(fb + 1) * P],
                    xt[:, e, qc2 * 512:(qc2 + 1) * 512],
                    start=(e == 0), stop=(e == 7),
                )
            nc.vector.tensor_scalar(
                QKT_sb[:, fb, qc * 512:(qc + 1) * 512], ps,
                (0.125 if fb < 2 else 1.0), qkb_sb[:, fb:fb + 1],
                ALU.mult, ALU.add,
            )
            return 4096
        return u

    def v_unit(st):
        def u():
            ps = ps_d.tile([P, 512], f32, tag="d", name="vps")
            pss = ps[:, :HPC * D]
            xt = xT_a if st < 8 else xT_b
            st2 = st if st < 8 else st - 8
            for e in range(8):
                nc.tensor.matmul(
                    pss, xt[:, e, st2 * P:(st2 + 1) * P], wvT_sb[:, e, :],
                    start=(e == 0), stop=False,
                )
            nc.tensor.matmul(pss, ones1, vb_sb, start=False, stop=True)
            nc.scalar.activation(
                V_sb[:, st, :, 0:D],
                pss.rearrange("p (h d) -> p h d", h=HPC),
                AF.Copy,
            )
            return 2048
        return u

    def sv_unit(pair, H, hh, qq, Ue):
        h = 2 * pair + hh
        kmax = 16 if H else 8
        nkj = min(kmax, (qq + 1) * 4)

        def u():
            Uq = ps_u.tile([D + 1, 512], f32, tag="u", name="Uq")
            cols = 0
            for kj in range(nkj):
                off, rs, rw = zoff[(H, kj)]
                a = max(rs, qq * 512)
                w = (qq + 1) * 512 - a
                cols += w
                nc.tensor.matmul(
                    Uq[:, a - qq * 512:a - qq * 512 + w],
                    V_sb[:, kj, h, :],
                    zbig[H][:, hh, off + a - rs:off + a - rs + w],
                    start=(kj == 0), stop=(kj == nkj - 1),
                )
            nc.vector.tensor_copy(Ue[hh][:, qq * 512:(qq + 1) * 512], Uq)
            return cols
        return u

    # batched output: accumulate 2 qt x 2 ec tiles in SBUF, 2 DMAs per group
    def out_units(pair, outT):
        units = []
        state = {}

        def mk(qt, ec):
            def u():
                if qt % 2 == 0 and ec == 0:
                    state['w0'] = osb.tile([P, 2, 512], bf16, tag="o0",
                                           name="ow0")
                    state['w1'] = osb.tile([P, 2, 512], bf16, tag="o1",
                                           name="ow1")
                po = ps_d.tile([P, 512], f32, tag="d", name="po")
                nc.tensor.matmul(
                    po,
                    AT_sb[:, pair, qt * P:(qt + 1) * P],
                    woT_sb[:, pair, ec * 512:(ec + 1) * 512],
                    start=True, stop=True,
                )
                wt = state['w0'] if ec == 0 else state['w1']
                if (qt + ec) % 2 == 0:
                    nc.vector.tensor_copy(wt[:, qt % 2, :], po)
                else:
                    nc.scalar.activation(wt[:, qt % 2, :], po, AF.Copy)
                if qt % 2 == 1 and ec == 1:
                    g = qt // 2
                    for e2, key in ((0, 'w0'), (1, 'w1')):
                        nc.sync.dma_start(outT[g, e2], state[key])
                return 512
            return u

        for qt in range(NQT):
            for ec in range(2):
                units.append(mk(qt, ec))
        return units

    # ---- z pass: transposed scores -> exp -> zbig slices (K=64 paired) ---
    # e^m max-folds are issued incrementally, one kj late, so they never
    # block the trl-mask -> exp chain on the DVE FIFO.
    def zpass(pair, H, ratio):
        q0 = H * 1024
        kmax = 16 if H else 8
        acc = acc_t[pair]
        pend = []
        with nc.named_scope(f"zpass_p{pair}h{H}"):
            drain(2048)
            for kj in range(kmax):
                off, rs, rw = zoff[(H, kj)]
                row_diag = (kj * P >= q0)
                pt = {}
                for hh in range(2):
                    pt[hh] = ps_s.tile([P, 1024], f32, tag="s",
                                       name="pt")[:, :rw]
                for ci in range(0, rw, 512):
                    cw = min(512, rw - ci)
                    for hh in range(2):
                        poff = D * hh
                        nc.tensor.matmul(
                            pt[hh][:, ci:ci + cw],
                            QKT_sb[poff:poff + D, 2 + pair,
                                   kj * P:(kj + 1) * P],
                            QKT_sb[poff:poff + D, pair,
                                   rs + ci:rs + ci + cw],
                            start=True, stop=True,
                        )
                for hh in range(2):
                    if row_diag:
                        nc.vector.tensor_tensor(
                            pt[hh][:, 0:P], pt[hh][:, 0:P],
                            trlf_sb, ALU.add,
                        )
                    nc.scalar.activation(
                        zbig[H][:, hh, off:off + rw], pt[hh], AF.Exp,
                    )

                def mkfold(kj, off, rs, rw):
                    def f():
                        for hh in range(2):
                            if kj == 0:
                                nc.vector.tensor_copy(
                                    acc[:, hh, rs:rs + rw],
                                    zbig[H][:, hh, off:off + rw])
                            else:
                                nc.vector.tensor_tensor(
                                    acc[:, hh, rs:rs + rw],
                                    acc[:, hh, rs:rs + rw],
                                    zbig[H][:, hh, off:off + rw], ALU.max,
                                )
                    return f
                pend.append(mkfold(kj, off, rs, rw))
                if len(pend) > 1:
                    pend.pop(0)()
                drain(int(ratio * rw))
            pend.pop(0)()

    def zpass_hh(pair, H, hh, ratio):
        # single-head z pass (serialized row group); used for the final
        # phase so hh=0's exps + stats chain complete while hh=1 runs
        q0 = H * 1024
        kmax = 16 if H else 8
        acc = acc_t[pair]
        poff = D * hh
        pend = []
        with nc.named_scope(f"zpassh_p{pair}h{H}_{hh}"):
            drain(2048)
            for kj in range(kmax):
                off, rs, rw = zoff[(H, kj)]
                row_diag = (kj * P >= q0)
                pt = ps_s.tile([P, 1024], f32, tag="s", name="pt")[:, :rw]
                for ci in range(0, rw, 512):
                    cw = min(512, rw - ci)
                    nc.tensor.matmul(
                        pt[:, ci:ci + cw],
                        QKT_sb[poff:poff + D, 2 + pair, kj * P:(kj + 1) * P],
                        QKT_sb[poff:poff + D, pair, rs + ci:rs + ci + cw],
                        start=True, stop=True,
                    )
                if row_diag:
                    nc.vector.tensor_tensor(
                        pt[:, 0:P], pt[:, 0:P], trlf_sb, ALU.add)
                nc.scalar.activation(
                    zbig[H][:, hh, off:off + rw], pt, AF.Exp)

                def mkfold(kj, off, rs, rw):
                    def f():
                        if kj == 0:
                            nc.vector.tensor_copy(
                                acc[:, hh, rs:rs + rw],
                                zbig[H][:, hh, off:off + rw])
                        else:
                            nc.vector.tensor_tensor(
                                acc[:, hh, rs:rs + rw],
                                acc[:, hh, rs:rs + rw],
                                zbig[H][:, hh, off:off + rw], ALU.max,
                            )
                    return f
                pend.append(mkfold(kj, off, rs, rw))
                if len(pend) > 1:
                    pend.pop(0)()
                drain(int(ratio * rw))
            pend.pop(0)()

    # ---- e^m extraction: gpsimd partition reduce over the fold result ----
    acc_t = {}
    par_t = {}

    def new_acc(pair):
        acc_t[pair] = accp.tile([P, 2, S], bf16, tag="acc", name="acc")
        par_t[pair] = parp.tile([P, 2, S], bf16, tag="par", name="par")

    def par(pair, half, heads=(0, 1)):
        a, b = (0, 1024) if half == 0 else (1024, 2048)
        for hh in heads:
            nc.gpsimd.partition_all_reduce(
                par_t[pair][:, hh, a:b], acc_t[pair][:, hh, a:b], P,
                bass_isa.ReduceOp.max,
            )

    def mtrans(pair, heads=(0, 1)):
        # e^m row (replicated) -> [128, 16] column layout via 16 tiny
        # transposes per head; bf16 psum columns padded to 4B alignment.
        cols = {}
        for hh in heads:
            emcol2 = ps_u.tile([P, 16, 2], bf16, tag="u", name="emcol2")
            for c in range(16):
                nc.tensor.transpose(
                    emcol2[:, c, 0:1],
                    par_t[pair][0:1, hh, c * P:(c + 1) * P],
                    id_bf[0:1, 0:1],
                )
            cols[hh] = emcol2
        return cols

    drow_tiles = {}

    def stats(pair, hh, Ue, emcol2):
        h = 2 * pair + hh
        if hh == 0:
            drow_tiles[pair] = drp.tile([D, 2, S], bf16, tag="drow",
                                        name="drow")
        with nc.named_scope(f"stats_h{h}"):
            # ghost column g * e^m
            ghost = spool.tile([P, 16], f32, tag="ghost", name="ghost")
            nc.scalar.activation(
                ghost, emcol2[:, :, 0], AF.Copy,
                scale=gco_sb[:, h:h + 1],
            )
            # sum-z row [1, 2048] -> column [128, 16] in psum
            szcol = ps_u.tile([P, 16], f32, tag="u", name="szcol")
            for c in range(16):
                nc.tensor.transpose(
                    szcol[:, c:c + 1], Ue[hh][D:D + 1, c * P:(c + 1) * P],
                    id_f[D:D + 1, D:D + 1],
                )
            denom = spool.tile([P, 16], f32, tag="denom", name="denom")
            nc.vector.tensor_tensor(denom, szcol, ghost, ALU.add)
            dcol = spool.tile([P, 16], f32, tag="dcol", name="dcol")
            nc.vector.reciprocal(dcol, denom)
            dT = ps_u.tile([16, P], f32, tag="u", name="dT")
            nc.tensor.transpose(dT, dcol, id_f)
            dT_sb = spool.tile([16, P], bf16, tag="dtsb", name="dT_sb")
            nc.vector.tensor_copy(dT_sb, dT)
            # reshape [16,128] -> [1,2048] via DRAM bounce
            dram_t = dscr.tile([16, P], bf16, tag="dbounce", name="dram_t")
            nc.sync.dma_start(dram_t, dT_sb)
            # replicate the reciprocal-denominator row across 64 partitions
            dslc = drow_tiles[pair][:, hh, :]
            nc.sync.dma_start(
                dslc,
                dram_t.rearrange("c w -> (c w)")[None, :].to_broadcast((D, S)),
            )
            # A^T[head] = U^T * drow, cast to bf16
            nc.vector.tensor_tensor(
                AT_sb[D * hh:D * hh + D, pair, :],
                Ue[hh][0:D, :],
                dslc,
                ALU.mult,
            )

    # ---- schedule -------------------------------------------------------
    RATIO = 3.0

    # pure-dense warmup: only the QKT blocks zpass(0,0) reads (coarse
    # dependency tracking makes its first matmul wait on the last QKT
    # write issued before it, so keep the warmup minimal)
    for qc in range(2):
        for fb in (0, 2):
            qkv_unit(fb, qc)()

    Ue0 = {hh: uep.tile([D + 1, S], f32, tag=f"ue{hh}", name="Ue")
           for hh in range(2)}

    for qc in range(2, 4):
        dense_q.append(qkv_unit(0, qc))
        dense_q.append(qkv_unit(2, qc))
    # pair-1 QKT next: its DVE copies must not trail the folds
    for qc in range(4):
        dense_q.append(qkv_unit(1, qc))
    for qc in range(4):
        dense_q.append(qkv_unit(3, qc))
    for st in range(NQT):
        dense_q.append(v_unit(st))
    new_acc(0)
    zpass(0, 0, RATIO)
    par(0, 0)

    for hh in range(2):
        for qq in (0, 1):
            dense_q.append(sv_unit(0, 0, hh, qq, Ue0))
    zpass(0, 1, RATIO)
    par(0, 1)
    drain_all()   # pair-1 QKT fully issued before pair-1 scores

    for hh in range(2):
        for qq in (2, 3):
            dense_q.append(sv_unit(0, 1, hh, qq, Ue0))
    new_acc(1)
    zpass(1, 0, RATIO)
    drain_all()

    em0 = mtrans(0)
    stats(0, 0, Ue0, em0[0])
    stats(0, 1, Ue0, em0[1])
    par(1, 0)

    Ue1 = {hh: uep.tile([D + 1, S], f32, tag=f"ue{hh}", name="Ue")
           for hh in range(2)}
    for hh in range(2):
        for qq in (0, 1):
            dense_q.append(sv_unit(1, 0, hh, qq, Ue1))
    zpass(1, 1, RATIO)
    par(1, 1)
    drain_all()

    # tail: pair-0 output projection is real dense work that keeps the HAM
    # at full clock while the last exps drain; then S@V + stats + out1.
    o0 = out_units(0, out0)
    for u in o0[0:14]:
        u()
    sv_unit(1, 1, 0, 2, Ue1)()
    for u in o0[14:20]:
        u()
    sv_unit(1, 1, 0, 3, Ue1)()
    for u in o0[20:24]:
        u()
    em1 = mtrans(1)
    for u in o0[24:28]:
        u()
    stats(1, 0, Ue1, em1[0])
    sv_unit(1, 1, 1, 2, Ue1)()
    for u in o0[28:32]:
        u()
    sv_unit(1, 1, 1, 3, Ue1)()
    stats(1, 1, Ue1, em1[1])
    heater(26)
    for u in out_units(1, out1):
        u()

    ctx.close()


def _host_inputs(inputs, Wqkv_w, Wqkv_b, Wo_w, ghost):
    """Build the 8 per-core input maps."""
    bf = ml_dtypes.bfloat16
    idf = np.eye(P, dtype=np.float32)
    idbf = np.eye(P, dtype=bf)
    trLf = (np.tril(np.ones((P, P), np.float32), -1) * NEG).astype(np.float32)
    in_maps = []
    for core in range(NCORES):
        b = core // 4
        g = core % 4
        r0 = g * HPC * D
        r1 = (g + 1) * HPC * D
        Wq = Wqkv_w[r0:r1]                      # [256, 1024]
        Wk = Wqkv_w[NHEAD * D + r0:NHEAD * D + r1]
        Wv = Wqkv_w[2 * NHEAD * D + r0:2 * NHEAD * D + r1]
        qk_bias = np.concatenate([Wqkv_b[r0:r1] / 8.0,
                                  Wqkv_b[NHEAD * D + r0:NHEAD * D + r1]])
        g_h = np.maximum(ghost[g * HPC:(g + 1) * HPC].astype(np.float64), 1e-38)
        in_maps.append({
            "xT": np.ascontiguousarray(inputs[b].T).astype(bf),
            "wqkT": np.ascontiguousarray(np.concatenate([Wq, Wk], 0).T).astype(bf),
            "wvT": np.ascontiguousarray(Wv.T).astype(bf),
            "woT": np.ascontiguousarray(Wo_w[:, r0:r1].T).astype(bf),
            "qkb": np.ascontiguousarray(
                qk_bias.reshape(4, P).T).astype(np.float32),
            "vb": Wqkv_b[2 * NHEAD * D + r0:2 * NHEAD * D + r1][None, :].astype(bf),
            "gco": np.tile(g_h.astype(np.float32)[None, :], (P, 1)),
            "idbf": idbf, "idf": idf, "trLf": trLf,
        })
    return in_maps


def kernel(inputs, Wqkv_w, Wqkv_b, Wo_w, Wo_b, ghost, _trace=False, _cores=NCORES):
    inputs = np.asarray(inputs, dtype=np.float32)
    Wqkv_w = np.asarray(Wqkv_w, dtype=np.float32)
    Wqkv_b = np.asarray(Wqkv_b, dtype=np.float32)
    Wo_w = np.asarray(Wo_w, dtype=np.float32)
    Wo_b = np.asarray(Wo_b, dtype=np.float32)
    ghost = np.asarray(ghost, dtype=np.float32)

    from concourse import bass_utils

    if "nc" not in _prog_cache:
        _prog_cache["nc"] = _build_program()
    nc = _prog_cache["nc"]

    in_maps = _host_inputs(inputs, Wqkv_w, Wqkv_b, Wo_w, ghost)
    res = bass_utils.run_bass_kernel_spmd(
        nc, in_maps[:_cores], core_ids=list(range(_cores)), trace=_trace,
    )
    full = np.zeros((B, S, EMBED), np.float32)
    for core in range(_cores):
        for key in ("out0", "out1"):
            o = res.results[core][key].astype(np.float32)  # [8,2,128,2,512]
            full[core // 4] += o.transpose(0, 3, 2, 1, 4).reshape(S, EMBED)
    full += Wo_b[None, None, :]
    if _trace:
        _prog_cache["last_results"] = res
    return full


# revision 43
# speedup vs baseline: 1.0087x; 1.0087x over previous
"""Trainium2 Bass kernel for nn_MultiHeadSelfAttention_22668837388979.

Sharding: 8 cores = batch(2) x head-groups(4).  Each core handles one batch
element and 4 of the 16 heads:
  - QKV projection (bf16 matmuls, f32 accum) for its heads
  - causal ghost-softmax attention
  - row-parallel output projection partials (one per head pair)
Host sums the per-core, per-pair partials and adds Wo_b.

Ghost softmax identity used on device (no max-subtraction pass needed):
  S = exp(s - m) / (sum exp(s - m) + g)  ==  z / (sum z + g * e^m),  z = exp(s)
e^m is recovered from the z tiles themselves: e^m = max_k z.  Per-head the
max over k is a cheap chain of bf16 DVE max-folds across the k-tile slices
(free axis) followed by one GpSimd partition_all_reduce (partition axis),
and 16 tiny PE transposes bring the row into [128,16] column layout for the
per-row denominator math.  sum z comes for free from a ones-column appended
to V in the S@V matmul.

Scheduling (from HAM throttle analysis): the PE clamps to 50% util whenever
the recent *instruction* mix is K=64-heavy (row-group pairing does not help
the HAM state, only throughput within it), and re-ramping costs ~20us.  So
every K=64 score matmul is issued interleaved with K=128 work (V proj, the
second half of QKV, S@V, output projection), the PE is pre-heated during
the initial input DMA, and DMAs are batched into few descriptors
(sync-engine issue costs ~0.65us each).  The z pipeline is gated by the
scalar engine's exp throughput, so the interleave ratio keeps the PE fed
with dense work while exps drain.
"""

import math
from collections import deque

import numpy as np
import ml_dtypes

EMBED = 1024
NHEAD = 16
D = 64
B = 2
S = 2048
HPC = 4          # heads per core
NCORES = 8
P = 128
NEG = -1.0e9
N_PRE = 60       # pre-heat matmuls at kernel start (PE idles during input DMA)

_prog_cache = {}


def _build_program():
    import concourse.bass as bass
    import concourse.tile as tile
    from concourse import bacc, mybir, bass_isa

    f32 = mybir.dt.float32
    bf16 = mybir.dt.bfloat16
    AF = mybir.ActivationFunctionType
    ALU = mybir.AluOpType
    AX = mybir.AxisListType

    nc = bacc.Bacc("TRN2", target_bir_lowering=False, debug=False)

    xT = nc.dram_tensor("xT", [EMBED, S], bf16, kind="ExternalInput").ap()
    wqkT = nc.dram_tensor("wqkT", [EMBED, 2 * HPC * D], bf16, kind="ExternalInput").ap()
    wvT = nc.dram_tensor("wvT", [EMBED, HPC * D], bf16, kind="ExternalInput").ap()
    woT = nc.dram_tensor("woT", [HPC * D, EMBED], bf16, kind="ExternalInput").ap()
    qkb = nc.dram_tensor("qkb", [P, 4], f32, kind="ExternalInput").ap()
    vb = nc.dram_tensor("vb", [1, HPC * D], bf16, kind="ExternalInput").ap()
    gco = nc.dram_tensor("gco", [P, HPC], f32, kind="ExternalInput").ap()
    idbf = nc.dram_tensor("idbf", [P, P], bf16, kind="ExternalInput").ap()
    idf = nc.dram_tensor("idf", [P, P], f32, kind="ExternalInput").ap()
    trLf = nc.dram_tensor("trLf", [P, P], f32, kind="ExternalInput").ap()
    out0 = nc.dram_tensor("out0", [S // P // 2, 2, P, 2, 512], bf16,
                          kind="ExternalOutput").ap()
    out1 = nc.dram_tensor("out1", [S // P // 2, 2, P, 2, 512], bf16,
                          kind="ExternalOutput").ap()

    with tile.TileContext(nc) as tc:
        _body(tc, bass, mybir, bass_isa, f32, bf16, AF, ALU, AX,
              xT, wqkT, wvT, woT, qkb, vb, gco, idbf, idf, trLf,
              out0, out1)

    nc.compile()
    return nc


def _body(tc, bass, mybir, bass_isa, f32, bf16, AF, ALU, AX,
          xT, wqkT, wvT, woT, qkb, vb, gco, idbf, idf, trLf,
          out0, out1):
    from contextlib import ExitStack
    nc = tc.nc
    NQT = S // P                 # 16 query tiles per head
    ctx = ExitStack()

    const = ctx.enter_context(tc.tile_pool(name="const", bufs=1))
    xpool = ctx.enter_context(tc.tile_pool(name="xpool", bufs=1))
    wpool = ctx.enter_context(tc.tile_pool(name="wpool", bufs=1))
    qkt = ctx.enter_context(tc.tile_pool(name="qkt", bufs=1))
    vsb = ctx.enter_context(tc.tile_pool(name="vsb", bufs=1))
    # z tiles: one big buffer per (pair, H-half); slices addressed by offset
    W_H0 = sum(1024 - 128 * kj for kj in range(8))              # 4608
    W_H1 = sum(min(1024, 2048 - 128 * kj) for kj in range(16))  # 12800
    zp0 = ctx.enter_context(tc.tile_pool(name="zp0", bufs=1))
    zp1 = ctx.enter_context(tc.tile_pool(name="zp1", bufs=1))
    accp = ctx.enter_context(tc.tile_pool(name="accp", bufs=1))
    parp = ctx.enter_context(tc.tile_pool(name="parp", bufs=1))
    spool = ctx.enter_context(tc.tile_pool(name="spool", bufs=2))
    at = ctx.enter_context(tc.tile_pool(name="at", bufs=1))
    osb = ctx.enter_context(tc.tile_pool(name="osb", bufs=2))
    dscr = ctx.enter_context(tc.tile_pool(name="dscr", bufs=2, space="DRAM"))

    uep = ctx.enter_context(tc.tile_pool(name="uep", bufs=1))
    drp = ctx.enter_context(tc.tile_pool(name="drp", bufs=1))

    ps_s = ctx.enter_context(tc.tile_pool(name="ps_s", bufs=2, space="PSUM"))
    ps_d = ctx.enter_context(tc.tile_pool(name="ps_d", bufs=2, space="PSUM"))
    ps_u = ctx.enter_context(tc.tile_pool(name="ps_u", bufs=2, space="PSUM"))

    def heater(n=16):
        """Dense K=128 matmul burst to keep/restore the PE HAM clock."""
        hp = ps_d.tile([P, 512], f32, tag="d", name="hp")
        for i in range(n):
            nc.tensor.matmul(hp, id_bf, wq_heat, start=True, stop=True)
        nc.vector.tensor_copy(heat_sink, hp[:, 0:1])

    # ---- constants built on-device (no DMA: pre-heat starts immediately) -
    tmp1b = const.tile([P, P], bf16)
    nc.vector.memset(tmp1b, 1.0)
    id_bf = const.tile([P, P], bf16)
    nc.gpsimd.affine_select(id_bf, tmp1b, [[1, P]], ALU.is_equal, 0.0,
                            base=0, channel_multiplier=-1)
    tmp1f = const.tile([P, P], f32)
    nc.vector.memset(tmp1f, 1.0)
    id_f = const.tile([P, P], f32)
    nc.gpsimd.affine_select(id_f, tmp1f, [[1, P]], ALU.is_equal, 0.0,
                            base=0, channel_multiplier=-1)
    tmp0f = const.tile([P, P], f32)
    nc.vector.memset(tmp0f, 0.0)
    trlf_sb = const.tile([P, P], f32)
    nc.gpsimd.affine_select(trlf_sb, tmp0f, [[1, P]], ALU.is_ge, NEG,
                            base=0, channel_multiplier=-1)
    ones1 = const.tile([1, P], bf16)
    nc.vector.memset(ones1, 1.0)
    heat_sink = const.tile([P, 1], f32)
    wq_heat = const.tile([P, 512], bf16)
    nc.vector.memset(wq_heat, 0.0)

    # pre-heat the PE while the big input DMAs stream in
    heater(N_PRE)

    qkb_sb = const.tile([P, 4], f32)
    nc.sync.dma_start(qkb_sb, qkb)
    vb_sb = const.tile([1, HPC * D], bf16)
    nc.sync.dma_start(vb_sb, vb)
    gco_sb = const.tile([P, HPC], f32)
    nc.sync.dma_start(gco_sb, gco)

    # ---- bulk inputs (single descriptor each; sync-engine issue is dear) -
    wqkT_sb = wpool.tile([P, 8, 2 * HPC * D], bf16)
    nc.sync.dma_start(wqkT_sb, wqkT.rearrange("(e p) c -> p e c", p=P))
    xT_a = xpool.tile([P, 8, S // 2], bf16, tag="xa", name="xT_a")
    nc.sync.dma_start(xT_a[:, 0:4, :],
                      xT[0:4 * P, 0:S // 2].rearrange("(e p) s -> p e s", p=P))
    nc.sync.dma_start(xT_a[:, 4:8, :],
                      xT[4 * P:, 0:S // 2].rearrange("(e p) s -> p e s", p=P))
    xT_b = xpool.tile([P, 8, S // 2], bf16, tag="xb", name="xT_b")
    nc.sync.dma_start(xT_b, xT[:, S // 2:].rearrange("(e p) s -> p e s", p=P))

    def xT_at(qc):
        return (xT_a if qc < 2 else xT_b), (qc if qc < 2 else qc - 2)
    wvT_sb = wpool.tile([P, 8, HPC * D], bf16)
    nc.sync.dma_start(wvT_sb, wvT.rearrange("(e p) c -> p e c", p=P))
    woT_sb = wpool.tile([P, 2, EMBED], bf16)
    nc.sync.dma_start(woT_sb, woT.rearrange("(b p) e -> p b e", p=P))

    QKT_sb = qkt.tile([P, 4, S], bf16)
    V_sb = vsb.tile([P, NQT, HPC, D + 1], bf16)
    nc.vector.memset(V_sb[:, :, :, D:D + 1], 1.0)
    AT_sb = at.tile([P, 2, S], bf16)   # A^T stacked: partitions = head%2*64+d
    zbig = {0: zp0.tile([P, 2, W_H0], bf16, name="zbig0"),
            1: zp1.tile([P, 2, W_H1], bf16, name="zbig1")}
    zoff = {}                     # (H, kj) -> (offset, rs, rw)
    for H in range(2):
        off = 0
        for kj in range(16 if H else 8):
            rs = max(kj * P, H * 1024)
            rw = H * 1024 + 1024 - rs
            zoff[(H, kj)] = (off, rs, rw)
            off += rw

    # ---- dense (K=128) unit machinery -----------------------------------
    dense_q = deque()

    def drain(cols):
        # Issue dense cover; if none is queued, burn a short K=128 heater
        # burst instead so the HAM never sees a K=64-only window.
        while cols > 0:
            if dense_q:
                cols -= dense_q.popleft()()
            else:
                hp = ps_d.tile([P, 512], f32, tag="d", name="hf")
                for _ in range(4):
                    nc.tensor.matmul(hp, id_bf, wq_heat, start=True, stop=True)
                nc.vector.tensor_copy(heat_sink, hp[:, 0:1])
                cols -= 2048

    def drain_all():
        while dense_q:
            dense_q.popleft()()

    def qkv_unit(fb, qc):
        def u():
            ps = ps_d.tile([P, 512], f32, tag="d", name="qkvps")
            xt, qc2 = xT_at(qc)
            for e in range(8):
                nc.tensor.matmul(
                    ps,
                    wqkT_sb[:, e, fb * P:(fb + 1) * P],
                    xt[:, e, qc2 * 512:(qc2 + 1) * 512],
                    start=(e == 0), stop=(e == 7),
                )
            nc.vector.tensor_scalar(
                QKT_sb[:, fb, qc * 512:(qc + 1) * 512], ps,
                (0.125 if fb < 2 else 1.0), qkb_sb[:, fb:fb + 1],
                ALU.mult, ALU.add,
            )
            return 4096
        return u

    def v_unit(st):
        def u():
            ps = ps_d.tile([P, 512], f32, tag="d", name="vps")
            pss = ps[:, :HPC * D]
            xt = xT_a if st < 8 else xT_b
            st2 = st if st < 8 else st - 8
            for e in range(8):
                nc.tensor.matmul(
                    pss, xt[:, e, st2 * P:(st2 + 1) * P], wvT_sb[:, e, :],
                    start=(e == 0), stop=False,
                )
            nc.tensor.matmul(pss, ones1, vb_sb, start=False, stop=True)
            nc.scalar.activation(
                V_sb[:, st, :, 0:D],
                pss.rearrange("p (h d) -> p h d", h=HPC),
                AF.Copy,
            )
            return 2048
        return u

    def sv_unit(pair, H, hh, qq, Ue):
        h = 2 * pair + hh
        kmax = 16 if H else 8
        nkj = min(kmax, (qq + 1) * 4)

        def u():
            Uq = ps_u.tile([D + 1, 512], f32, tag="u", name="Uq")
            cols = 0
            for kj in range(nkj):
                off, rs, rw = zoff[(H, kj)]
                a = max(rs, qq * 512)
                w = (qq + 1) * 512 - a
                cols += w
                nc.tensor.matmul(
                    Uq[:, a - qq * 512:a - qq * 512 + w],
                    V_sb[:, kj, h, :],
                    zbig[H][:, hh, off + a - rs:off + a - rs + w],
                    start=(kj == 0), stop=(kj == nkj - 1),
                )
            nc.vector.tensor_copy(Ue[hh][:, qq * 512:(qq + 1) * 512], Uq)
            return cols
        return u

    # batched output: accumulate 2 qt x 2 ec tiles in SBUF, 2 DMAs per group
    def out_units(pair, outT):
        units = []
        state = {}

        def mk(qt, ec):
            def u():
                if qt % 2 == 0 and ec == 0:
                    state['w0'] = osb.tile([P, 2, 512], bf16, tag="o0",
                                           name="ow0")
                    state['w1'] = osb.tile([P, 2, 512], bf16, tag="o1",
                                           name="ow1")
                po = ps_d.tile([P, 512], f32, tag="d", name="po")
                nc.tensor.matmul(
                    po,
                    AT_sb[:, pair, qt * P:(qt + 1) * P],
                    woT_sb[:, pair, ec * 512:(ec + 1) * 512],
                    start=True, stop=True,
                )
                wt = state['w0'] if ec == 0 else state['w1']
                if (qt + ec) % 2 == 0:
                    nc.vector.tensor_copy(wt[:, qt % 2, :], po)
                else:
                    nc.scalar.activation(wt[:, qt % 2, :], po, AF.Copy)
                if qt % 2 == 1 and ec == 1:
                    g = qt // 2
                    for e2, key in ((0, 'w0'), (1, 'w1')):
                        nc.sync.dma_start(outT[g, e2], state[key])
                return 512
            return u

        for qt in range(NQT):
            for ec in range(2):
                units.append(mk(qt, ec))
        return units

    # ---- z pass: transposed scores -> exp -> zbig slices (K=64 paired) ---
    # e^m max-folds are issued incrementally, one kj late, so they never
    # block the trl-mask -> exp chain on the DVE FIFO.
    def zpass(pair, H, ratio):
        q0 = H * 1024
        kmax = 16 if H else 8
        acc = acc_t[pair]
        pend = []
        with nc.named_scope(f"zpass_p{pair}h{H}"):
            drain(2048)
            for kj in range(kmax):
                off, rs, rw = zoff[(H, kj)]
                row_diag = (kj * P >= q0)
                pt = {}
                for hh in range(2):
                    pt[hh] = ps_s.tile([P, 1024], f32, tag="s",
                                       name="pt")[:, :rw]
                for ci in range(0, rw, 512):
                    cw = min(512, rw - ci)
                    for hh in range(2):
                        poff = D * hh
                        nc.tensor.matmul(
                            pt[hh][:, ci:ci + cw],
                            QKT_sb[poff:poff + D, 2 + pair,
                                   kj * P:(kj + 1) * P],
                            QKT_sb[poff:poff + D, pair,
                                   rs + ci:rs + ci + cw],
                            start=True, stop=True,
                        )
                for hh in range(2):
                    if row_diag:
                        nc.vector.tensor_tensor(
                            pt[hh][:, 0:P], pt[hh][:, 0:P],
                            trlf_sb, ALU.add,
                        )
                    nc.scalar.activation(
                        zbig[H][:, hh, off:off + rw], pt[hh], AF.Exp,
                    )

                def mkfold(kj, off, rs, rw):
                    def f():
                        for hh in range(2):
                            if kj == 0:
                                nc.vector.tensor_copy(
                                    acc[:, hh, rs:rs + rw],
                                    zbig[H][:, hh, off:off + rw])
                            else:
                                nc.vector.tensor_tensor(
                                    acc[:, hh, rs:rs + rw],
                                    acc[:, hh, rs:rs + rw],
                                    zbig[H][:, hh, off:off + rw], ALU.max,
                                )
                    return f
                pend.append(mkfold(kj, off, rs, rw))
                if len(pend) > 1:
                    pend.pop(0)()
                drain(int(ratio * rw))
            pend.pop(0)()

    def zpass_hh(pair, H, hh, ratio):
        # single-head z pass (serialized row group); used for the final
        # phase so hh=0's exps + stats chain complete while hh=1 runs
        q0 = H * 1024
        kmax = 16 if H else 8
        acc = acc_t[pair]
        poff = D * hh
        pend = []
        with nc.named_scope(f"zpassh_p{pair}h{H}_{hh}"):
            drain(2048)
            for kj in range(kmax):
                off, rs, rw = zoff[(H, kj)]
                row_diag = (kj * P >= q0)
                pt = ps_s.tile([P, 1024], f32, tag="s", name="pt")[:, :rw]
                for ci in range(0, rw, 512):
                    cw = min(512, rw - ci)
                    nc.tensor.matmul(
                        pt[:, ci:ci + cw],
                        QKT_sb[poff:poff + D, 2 + pair, kj * P:(kj + 1) * P],
                        QKT_sb[poff:poff + D, pair, rs + ci:rs + ci + cw],
                        start=True, stop=True,
                    )
                if row_diag:
                    nc.vector.tensor_tensor(
                        pt[:, 0:P], pt[:, 0:P], trlf_sb, ALU.add)
                nc.scalar.activation(
                    zbig[H][:, hh, off:off + rw], pt, AF.Exp)

                def mkfold(kj, off, rs, rw):
                    def f():
                        if kj == 0:
                            nc.vector.tensor_copy(
                                acc[:, hh, rs:rs + rw],
                                zbig[H][:, hh, off:off + rw])
                        else:
                            nc.vector.tensor_tensor(
                                acc[:, hh, rs:rs + rw],
                                acc[:, hh, rs:rs + rw],
                                zbig[H][:, hh, off:off + rw], ALU.max,
                            )
                    return f
                pend.append(mkfold(kj, off, rs, rw))
                if len(pend) > 1:
                    pend.pop(0)()
                drain(int(ratio * rw))
            pend.pop(0)()

    # ---- e^m extraction: gpsimd partition reduce over the fold result ----
    acc_t = {}
    par_t = {}

    def new_acc(pair):
        acc_t[pair] = accp.tile([P, 2, S], bf16, tag="acc", name="acc")
        par_t[pair] = parp.tile([P, 2, S], bf16, tag="par", name="par")

    def par(pair, half, heads=(0, 1)):
        a, b = (0, 1024) if half == 0 else (1024, 2048)
        for hh in heads:
            nc.gpsimd.partition_all_reduce(
                par_t[pair][:, hh, a:b], acc_t[pair][:, hh, a:b], P,
                bass_isa.ReduceOp.max,
            )

    def mtrans(pair, heads=(0, 1)):
        # e^m row (replicated) -> [128, 16] column layout via 16 tiny
        # transposes per head; bf16 psum columns padded to 4B alignment.
        cols = {}
        for hh in heads:
            emcol2 = ps_u.tile([P, 16, 2], bf16, tag="u", name="emcol2")
            for c in range(16):
                nc.tensor.transpose(
                    emcol2[:, c, 0:1],
                    par_t[pair][0:1, hh, c * P:(c + 1) * P],
                    id_bf[0:1, 0:1],
                )
            cols[hh] = emcol2
        return cols

    drow_tiles = {}

    def stats(pair, hh, Ue, emcol2):
        h = 2 * pair + hh
        if hh == 0:
            drow_tiles[pair] = drp.tile([D, 2, S], bf16, tag="drow",
                                        name="drow")
        with nc.named_scope(f"stats_h{h}"):
            # ghost column g * e^m
            ghost = spool.tile([P, 16], f32, tag="ghost", name="ghost")
            nc.scalar.activation(
                ghost, emcol2[:, :, 0], AF.Copy,
                scale=gco_sb[:, h:h + 1],
            )
            # sum-z row [1, 2048] -> column [128, 16] in psum
            szcol = ps_u.tile([P, 16], f32, tag="u", name="szcol")
            for c in range(16):
                nc.tensor.transpose(
                    szcol[:, c:c + 1], Ue[hh][D:D + 1, c * P:(c + 1) * P],
                    id_f[D:D + 1, D:D + 1],
                )
            denom = spool.tile([P, 16], f32, tag="denom", name="denom")
            nc.vector.tensor_tensor(denom, szcol, ghost, ALU.add)
            dcol = spool.tile([P, 16], f32, tag="dcol", name="dcol")
            nc.vector.reciprocal(dcol, denom)
            dT = ps_u.tile([16, P], f32, tag="u", name="dT")
            nc.tensor.transpose(dT, dcol, id_f)
            dT_sb = spool.tile([16, P], bf16, tag="dtsb", name="dT_sb")
            nc.vector.tensor_copy(dT_sb, dT)
            # reshape [16,128] -> [1,2048] via DRAM bounce
            dram_t = dscr.tile([16, P], bf16, tag="dbounce", name="dram_t")
            nc.sync.dma_start(dram_t, dT_sb)
            # replicate the reciprocal-denominator row across 64 partitions
            dslc = drow_tiles[pair][:, hh, :]
            nc.sync.dma_start(
                dslc,
                dram_t.rearrange("c w -> (c w)")[None, :].to_broadcast((D, S)),
            )
            # A^T[head] = U^T * drow, cast to bf16
            nc.vector.tensor_tensor(
                AT_sb[D * hh:D * hh + D, pair, :],
                Ue[hh][0:D, :],
                dslc,
                ALU.mult,
            )

    # ---- schedule -------------------------------------------------------
    RATIO = 3.0

    # pure-dense warmup: only the QKT blocks zpass(0,0) reads (coarse
    # dependency tracking makes its first matmul wait on the last QKT
    # write issued before it, so keep the warmup minimal)
    for qc in range(2):
        for fb in (0, 2):
            qkv_unit(fb, qc)()

    Ue0 = {hh: uep.tile([D + 1, S], f32, tag=f"ue{hh}", name="Ue")
           for hh in range(2)}

    for qc in range(2, 4):
        dense_q.append(qkv_unit(0, qc))
        dense_q.append(qkv_unit(2, qc))
    # pair-1 QKT next: its DVE copies must not trail the folds
    for qc in range(4):
        dense_q.append(qkv_unit(1, qc))
    for qc in range(4):
        dense_q.append(qkv_unit(3, qc))
    for st in range(NQT):
        dense_q.append(v_unit(st))
    new_acc(0)
    zpass(0, 0, RATIO)
    par(0, 0)

    for hh in range(2):
        for qq in (0, 1):
            dense_q.append(sv_unit(0, 0, hh, qq, Ue0))
    zpass(0, 1, RATIO)
    par(0, 1)
    drain_all()   # pair-1 QKT fully issued before pair-1 scores

    for hh in range(2):
        for qq in (2, 3):
            dense_q.append(sv_unit(0, 1, hh, qq, Ue0))
    new_acc(1)
    zpass(1, 0, RATIO)
    drain_all()

    em0 = mtrans(0)
    stats(0, 0, Ue0, em0[0])
    heater(6)
    stats(0, 1, Ue0, em0[1])
    par(1, 0)
    heater(8)

    Ue1 = {hh: uep.tile([D + 1, S], f32, tag=f"ue{hh}", name="Ue")
           for hh in range(2)}
    for hh in range(2):
        for qq in (0, 1):
            dense_q.append(sv_unit(1, 0, hh, qq, Ue1))
    zpass(1, 1, RATIO)
    par(1, 1)
    drain_all()

    # tail: pair-0 output projection is real dense work that keeps the HAM
    # at full clock while the last exps drain; then S@V + stats + out1.
    o0 = out_units(0, out0)
    for u in o0[0:14]:
        u()
    sv_unit(1, 1, 0, 2, Ue1)()
    for u in o0[14:20]:
        u()
    sv_unit(1, 1, 0, 3, Ue1)()
    for u in o0[20:24]:
        u()
    em1 = mtrans(1)
    for u in o0[24:28]:
        u()
    stats(1, 0, Ue1, em1[0])
    sv_unit(1, 1, 1, 2, Ue1)()
    for u in o0[28:32]:
        u()
    sv_unit(1, 1, 1, 3, Ue1)()
    stats(1, 1, Ue1, em1[1])
    heater(26)
    for u in out_units(1, out1):
        u()

    ctx.close()


def _host_inputs(inputs, Wqkv_w, Wqkv_b, Wo_w, ghost):
    """Build the 8 per-core input maps."""
    bf = ml_dtypes.bfloat16
    idf = np.eye(P, dtype=np.float32)
    idbf = np.eye(P, dtype=bf)
    trLf = (np.tril(np.ones((P, P), np.float32), -1) * NEG).astype(np.float32)
    in_maps = []
    for core in range(NCORES):
        b = core // 4
        g = core % 4
        r0 = g * HPC * D
        r1 = (g + 1) * HPC * D
        Wq = Wqkv_w[r0:r1]                      # [256, 1024]
        Wk = Wqkv_w[NHEAD * D + r0:NHEAD * D + r1]
        Wv = Wqkv_w[2 * NHEAD * D + r0:2 * NHEAD * D + r1]
        qk_bias = np.concatenate([Wqkv_b[r0:r1] / 8.0,
                                  Wqkv_b[NHEAD * D + r0:NHEAD * D + r1]])
        g_h = np.maximum(ghost[g * HPC:(g + 1) * HPC].astype(np.float64), 1e-38)
        in_maps.append({
            "xT": np.ascontiguousarray(inputs[b].T).astype(bf),
            "wqkT": np.ascontiguousarray(np.concatenate([Wq, Wk], 0).T).astype(bf),
            "wvT": np.ascontiguousarray(Wv.T).astype(bf),
            "woT": np.ascontiguousarray(Wo_w[:, r0:r1].T).astype(bf),
            "qkb": np.ascontiguousarray(
                qk_bias.reshape(4, P).T).astype(np.float32),
            "vb": Wqkv_b[2 * NHEAD * D + r0:2 * NHEAD * D + r1][None, :].astype(bf),
            "gco": np.tile(g_h.astype(np.float32)[None, :], (P, 1)),
            "idbf": idbf, "idf": idf, "trLf": trLf,
        })
    return in_maps


def kernel(inputs, Wqkv_w, Wqkv_b, Wo_w, Wo_b, ghost, _trace=False, _cores=NCORES):
    inputs = np.asarray(inputs, dtype=np.float32)
    Wqkv_w = np.asarray(Wqkv_w, dtype=np.float32)
    Wqkv_b = np.asarray(Wqkv_b, dtype=np.float32)
    Wo_w = np.asarray(Wo_w, dtype=np.float32)
    Wo_b = np.asarray(Wo_b, dtype=np.float32)
    ghost = np.asarray(ghost, dtype=np.float32)

    from concourse import bass_utils

    if "nc" not in _prog_cache:
        _prog_cache["nc"] = _build_program()
    nc = _prog_cache["nc"]

    in_maps = _host_inputs(inputs, Wqkv_w, Wqkv_b, Wo_w, ghost)
    res = bass_utils.run_bass_kernel_spmd(
        nc, in_maps[:_cores], core_ids=list(range(_cores)), trace=_trace,
    )
    full = np.zeros((B, S, EMBED), np.float32)
    for core in range(_cores):
        for key in ("out0", "out1"):
            o = res.results[core][key].astype(np.float32)  # [8,2,128,2,512]
            full[core // 4] += o.transpose(0, 3, 2, 1, 4).reshape(S, EMBED)
    full += Wo_b[None, None, :]
    if _trace:
        _prog_cache["last_results"] = res
    return full
